# revision 1
# baseline (speedup 1.0000x reference)
"""Trainium2 Bass kernel for nn_AssignAttention (hard-assignment MoE-routing attention).

Math (forward): for each (b, h, key-token s), the key token is hard-assigned to
group n* = argmax_n (q_bhn . k_bhs); output per group = sum of assigned v vectors
scaled by 1/(count+1), then projected.  The straight-through softmax terms cancel
in forward up to ~1e-7, so only the argmax routing matters.

Strategy:
 - Pure data-parallel over batch B=16 across 8 cores (2 batches/core), no collectives.
 - Host precomputes t[b,h,n,:] = Wk_h^T Wq_h query[b,n] so attention logits are
   attn[s, (h,n)] = key[b,s,:] . t[b,h,n,:]  -- one C-contraction against raw key.
 - Host pre-transposes key to keyT [C, S]; all transfers use the (ct p) x ->
   p ct x rearrange, whose ~2KB-per-descriptor granularity measures fastest on
   the DMA queues (~23GB/s/queue; 6KB descriptors measured slower).
 - Attention logits use float32r matmuls (1 cyc/row, ~13-bit mantissa): measured
   argmax flip-induced error ~0.008 rel, well within tolerance. v/output paths in
   float32r/bf16.
 - Per 128-row s-subtile: argmax over each head's 64 logit columns (free-axis
   reduce_max + one broadcast is_equal -> bf16 one-hot on DVE), then
   PSUM-accumulate head-PAIR-packed o += aT_pair^T @ [v|1|v|1] (the ones column
   yields per-group counts; it is written once per rotating buffer, not per
   subtile).  The o-matmuls are flushed in one burst per CHUNK (after the next
   chunk's first subtile's attn/v): the PE pays its f32r<->bf16 reconfiguration
   penalty (~55ns) twice per burst instead of twice per subtile, and the extra
   pipeline depth keeps it off DVE's critical path.
 - Startup: the NEFF boot blocks all sequencers ~7.5us and each DMA trigger
   costs ~0.7us of sequencer time, so kt-chunk-0 plus batch 0's whole tc ride
   in ONE merged transfer (one trigger, one completion; 2KB rows), chunks ramp
   128/128/256/512..., and warmup matmuls absorb the PE pstate ramp while that
   transfer lands.
 - Epilogue scales by 1/(cnt+1) straight out of PSUM with two stride-0
   broadcast multiplies, transposes via PE, projects, and DMAs out from the
   Activation queue (no cross-engine hop after the copy that produced it).
   (An all-f32r epilogue to avoid dtype-switches when its instructions
   interleave with the next batch's attention stream fails to compile:
   walrus rejects f32r transposes/memsets.)
"""
import sys

sys.path.insert(0, "/opt/trn_rl_repo")

import numpy as np
import ml_dtypes

import concourse.bass as bass
import concourse.mybir as mybir
import concourse.tile as tile
from concourse.bass_utils import run_bass_kernel_spmd
from concourse.masks import make_identity

B, N, S, C, H = 16, 64, 4096, 384, 6
DH = C // H  # 64
NCORES = 8
BPC = B // NCORES  # batches per core = 2
CT = C // 128  # c-tiles = 3
# chunk boundaries: tiny leading chunks so the DMA pipeline can feed the PE
# as soon as the merged first transfer lands, then 512-token chunks (1024
# starves the early DMA pipeline; an extra-small final chunk costs more in
# added boundaries than its shorter tail wait saves)
CHUNK_BOUNDS = [0, 128, 256, 512] + list(range(1024, S, 512)) + [S]
CHUNKS = list(zip(CHUNK_BOUNDS[:-1], CHUNK_BOUNDS[1:]))

F32 = mybir.dt.float32
F32R = mybir.dt.float32r
BF16 = mybir.dt.bfloat16

LAST_RESULT = None  # stash of BassKernelResults for profiling in test.py


def _split_multiwaits(nc):
    """walrus codegen in this toolchain accepts at most one sync-wait per
    instruction; hoist extras onto standalone wait-only EventSemaphore
    instructions placed immediately before (same engine, so ordering holds)."""
    for fn in nc.m.functions:
        for blk in fn.blocks:
            new = []
            for inst in blk.instructions:
                si = inst.sync_info
                if si is not None and si.on_wait and len(si.on_wait) > 1:
                    for w in si.on_wait[:-1]:
                        ev = mybir.InstEventSemaphore(
                            name=nc.get_next_instruction_name(), ins=[], outs=[]
                        )
                        ev.engine = inst.engine
                        ev.sync_info = mybir.SyncInfo(on_wait=[w], on_update=[])
                        new.append(ev)
                    inst.sync_info = mybir.SyncInfo(
                        on_wait=[si.on_wait[-1]], on_update=si.on_update
                    )
                new.append(inst)
            blk.instructions = new


def _build_kernel():
    nc = bass.Bass()
    # pre: merged [kt chunk0 | tc] for batch 0; row (ct*128+p) = [key tokens
    # 0:128 | tc columns] of c-row ct*128+p, so each (p, ct) descriptor is 2KB
    pre_d = nc.declare_dram_parameter("pre", [C, 128 + C], F32R, isOutput=False)
    keyT_d = nc.declare_dram_parameter("keyT", [BPC, C, S], F32R, isOutput=False)
    tc_d = nc.declare_dram_parameter("tc", [BPC, C, C], F32R, isOutput=False)
    wvt_d = nc.declare_dram_parameter("wvt", [C, C], F32R, isOutput=False)
    wpt_d = nc.declare_dram_parameter("wpt", [C, C], BF16, isOutput=False)
    out_d = nc.declare_dram_parameter("out", [BPC, N, C], F32, isOutput=True)

    with tile.TileContext(nc) as tc:
        with (
            tc.tile_pool(name="consts", bufs=1) as consts,
            tc.tile_pool(name="perb", bufs=2) as perb,
            tc.tile_pool(name="keyp", bufs=6) as keyp,
            tc.tile_pool(name="work", bufs=1) as work,
            tc.tile_pool(name="epi", bufs=2) as epi,
            tc.tile_pool(name="ps_attn", bufs=4, space="PSUM") as ps_attn,
            tc.tile_pool(name="ps_v", bufs=2, space="PSUM") as ps_v,
            tc.tile_pool(name="ps_o", bufs=1, space="PSUM") as ps_o,
            tc.tile_pool(name="ps_epi", bufs=1, space="PSUM") as ps_epi,
        ):
            # one merged transfer delivers everything subtile 0 needs
            pre_sb = consts.tile([128, CT, 128 + C], F32R)
            nc.sync.dma_start(
                out=pre_sb[:],
                in_=pre_d.rearrange("(ct p) x -> p ct x", p=128),
            )
            kt_c0 = pre_sb[:, :, 0:128]
            tc_b0 = pre_sb[:, :, 128 : 128 + C]
            wvt_sb = consts.tile([128, CT, C], F32R)  # [c_in_p, ct, c_out]
            nc.sync.dma_start(
                out=wvt_sb[:], in_=wvt_d.rearrange("(ct p) co -> p ct co", p=128)
            )
            keyT_b0 = keyT_d[0].rearrange("(ct p) s -> p ct s", p=128)
            s0, s1 = CHUNKS[1]
            kt_c1 = keyp.tile([128, CT, s1 - s0], F32R, tag="kt")
            nc.sync.dma_start(out=kt_c1[:], in_=keyT_b0[:, :, s0:s1])
            s0, s1 = CHUNKS[2]
            kt_c2 = keyp.tile([128, CT, s1 - s0], F32R, tag="kt")
            nc.sync.dma_start(out=kt_c2[:], in_=keyT_b0[:, :, s0:s1])
            # wpt is needed only at the first epilogue (~40us in), so its
            # transfer is deferred out of the latency-critical early DMA FIFO
            # (trigger emitted after chunk 5's, inside the loop)
            wpt_sb = consts.tile([128, CT, C], BF16)  # [hd_p, ct, c_out]
            # two stacked 64x64 identities so transposes of partition-offset-64
            # slices have a matching-base-partition rhs
            ident2 = consts.tile([128, N], BF16)
            make_identity(nc, ident2[0:N, :])
            make_identity(nc, ident2[N : 2 * N, :])

            # PE warmup: back-to-back matmuls on scratch while the first
            # transfer lands, so the pstate ramp completes before real work.
            # The psum bank is never read; its reuse starts with start=True.
            warm_sb = consts.tile([128, 640], BF16)
            nc.gpsimd.memset(warm_sb[:], 0.0)
            warm_ps = ps_attn.tile([128, 512], F32, tag="attn_ps")
            for _ in range(10):
                nc.tensor.matmul(
                    warm_ps[:], warm_sb[:, 0:128], warm_sb[:, 128:640],
                    start=True, stop=True,
                )

            # v65 ring: the ones column (counts) is written once per buffer;
            # the per-subtile copy only rewrites the v lanes
            v65_ring = [
                work.tile([128, H, DH + 1], BF16, tag=f"v65_{i}", name=f"v65_{i}")
                for i in range(12)
            ]
            for t in v65_ring:
                nc.gpsimd.memset(t[:, :, DH : DH + 1], 1.0)

            sub_ctr = 0
            for b in range(BPC):
                if b == 0:
                    tc_sb = tc_b0
                else:
                    tc_t = perb.tile([128, CT, C], F32R, tag="tc_sb")
                    nc.sync.dma_start(
                        out=tc_t[:],
                        in_=tc_d[b].rearrange("(ct p) hn -> p ct hn", p=128),
                    )
                    tc_sb = tc_t[:, :, :]
                # per-group accumulator, head-PAIR packed: for pair p, partition
                # rows 0..63 = head 2p groups, rows 64..127 = head 2p+1 groups;
                # col 64 = counts for both heads; cols 0..63 / 65..128 hold the
                # two heads' v-sums (off-diagonal blocks are junk, never read).
                # Zeroed explicitly; the accumulating matmuls use start=False so
                # their order doesn't matter (add-or-overwrite onto zeros commutes).
                o_ps = ps_o.tile([128, CT, 2 * DH + 2], F32)
                nc.vector.memset(o_ps[:], 0.0)

                keyT_b = keyT_d[b].rearrange("(ct p) s -> p ct s", p=128)
                # o-matmuls are flushed one chunk at a time, after the NEXT
                # chunk's first subtile's attn/v (see module docstring)
                pending = []  # [(aT, v65), ...] of the previous chunk

                def flush_o(stop):
                    for i, (aT_p, v65_p) in enumerate(pending):
                        last_sub = i == len(pending) - 1
                        for p in range(CT):
                            nc.tensor.matmul(
                                o_ps[:, p, :],
                                aT_p[:].rearrange("q h n -> q (h n)")[
                                    :, 2 * p * N : (2 * p + 2) * N
                                ],
                                v65_p[:].rearrange("q h d -> q (h d)")[
                                    :, 2 * p * (DH + 1) : (2 * p + 2) * (DH + 1)
                                ],
                                start=False,
                                stop=stop and last_sub and p == CT - 1,
                                skip_group_check=True,
                            )
                    pending.clear()

                for ci, (s0, s1) in enumerate(CHUNKS):
                    if b == 0 and ci == 0:
                        kt_sb = kt_c0
                    elif b == 0 and ci == 1:
                        kt_sb = kt_c1[:, :, :]
                    elif b == 0 and ci == 2:
                        kt_sb = kt_c2[:, :, :]
                    else:
                        kt_t = keyp.tile([128, CT, s1 - s0], F32R, tag="kt")
                        nc.sync.dma_start(
                            out=kt_t[:], in_=keyT_b[:, :, s0:s1]
                        )
                        kt_sb = kt_t[:, :, :]
                        if b == 0 and ci == 5:
                            nc.sync.dma_start(
                                out=wpt_sb[:],
                                in_=wpt_d.rearrange("(ct p) co -> p ct co", p=128),
                            )
                    carry = []
                    for sub in range((s1 - s0) // 128):
                        sl = slice(sub * 128, (sub + 1) * 128)
                        attn_ps = ps_attn.tile([128, C], F32)
                        v_ps = ps_v.tile([128, C], F32)
                        # all attn matmuls first so the logit group closes
                        # ~3 matmuls earlier and DVE's argmax starts sooner
                        for ct in range(CT):
                            nc.tensor.matmul(
                                attn_ps[:],
                                kt_sb[:, ct, sl],
                                tc_sb[:, ct, :],
                                start=(ct == 0),
                                stop=(ct == CT - 1),
                            )
                        for ct in range(CT):
                            nc.tensor.matmul(
                                v_ps[:],
                                kt_sb[:, ct, sl],
                                wvt_sb[:, ct, :],
                                start=(ct == 0),
                                stop=(ct == CT - 1),
                            )
                        if sub == min(1, (s1 - s0) // 128 - 1) and pending:
                            # flush the previous chunk's o-burst one subtile
                            # later than strictly needed: the extra subtile of
                            # DVE slack hides the last one-hot's latency so
                            # the burst never stalls on entry
                            flush_o(stop=False)
                        # per-head argmax -> one-hot (bf16); both ops read
                        # PSUM so they must stay on DVE (GpSimd/Pool cannot
                        # access PSUM)
                        gmax = work.tile([128, H], F32, tag="gmax", bufs=4)
                        nc.vector.reduce_max(
                            out=gmax[:],
                            in_=attn_ps[:].rearrange("p (h n) -> p h n", h=H),
                            axis=mybir.AxisListType.X,
                        )
                        aT = work.tile([128, H, N], BF16, tag="aT", bufs=12)
                        g = gmax[:]
                        g_bcast = bass.AP(
                            tensor=g.tensor, offset=g.offset,
                            ap=[g.ap[0], g.ap[1], [0, N]],
                        )
                        nc.vector.tensor_tensor(
                            out=aT[:],
                            in0=attn_ps[:].rearrange("p (h n) -> p h n", h=H),
                            in1=g_bcast,
                            op=mybir.AluOpType.is_equal,
                        )
                        # v lanes (bf16); the ones column is already in place
                        v65 = v65_ring[sub_ctr % 12]
                        sub_ctr += 1
                        nc.scalar.copy(
                            out=v65[:, :, 0:DH],
                            in_=v_ps[:].rearrange("p (h d) -> p h d", h=H),
                        )
                        carry.append((aT, v65))
                    pending.extend(carry)
                last_aT = pending[-1][0]
                flush_o(stop=True)
                if b == BPC - 1:
                    # fill the PE's wait on DVE's scaling chain with scratch
                    # matmuls so its clock stays in the top p-state for the
                    # tail transposes/projection.  Reading the final one-hot
                    # keeps the scheduler from hoisting them earlier, and
                    # bf16 operands right after the bf16 o-burst add no
                    # dtype-switch.
                    warm_w = last_aT[:].rearrange("q h n -> q (h n)")[:, 0:128]
                    for _ in range(3):
                        nc.tensor.matmul(
                            warm_ps[:], warm_w, warm_sb[:, 128:640],
                            start=True, stop=True,
                        )
                # epilogue for this b: scale by 1/(cnt+1) (cnt in col 64 for
                # both heads of each pair) straight out of PSUM -- two
                # stride-0-broadcast multiplies -- then transpose to [hd, n],
                # project, and DMA out
                scl = epi.tile([128, CT], F32)
                nc.vector.tensor_scalar(
                    out=scl[:],
                    in0=o_ps[:, :, DH],
                    scalar1=1.0,
                    scalar2=None,
                    op0=mybir.AluOpType.add,
                )
                nc.vector.reciprocal(out=scl[:], in_=scl[:])
                osc = epi.tile([128, CT, DH], BF16)
                s0_ = scl[0:N, :]
                s0b = bass.AP(
                    tensor=s0_.tensor, offset=s0_.offset,
                    ap=[s0_.ap[0], s0_.ap[1], [0, DH]],
                )
                nc.vector.tensor_tensor(
                    out=osc[0:N, :, :],
                    in0=o_ps[0:N, :, 0:DH],
                    in1=s0b,
                    op=mybir.AluOpType.mult,
                )
                s1_ = scl[N : 2 * N, :]
                s1b = bass.AP(
                    tensor=s1_.tensor, offset=s1_.offset,
                    ap=[s1_.ap[0], s1_.ap[1], [0, DH]],
                )
                nc.vector.tensor_tensor(
                    out=osc[N : 2 * N, :, :],
                    in0=o_ps[N : 2 * N, :, DH + 1 : 2 * DH + 1],
                    in1=s1b,
                    op=mybir.AluOpType.mult,
                )
                # osc[0:64, p, :] = [n, dh] of head 2p -> oT rows 128p+dh;
                # osc[64:128, p, :] = [n, dh] of head 2p+1 -> oT rows 128p+64+dh
                oT_ps = ps_epi.tile([128, CT, N], BF16, tag="epi_ps")
                for p in range(CT):
                    nc.tensor.transpose(
                        oT_ps[0:N, p, :], osc[0:N, p, :], ident2[0:N, :]
                    )
                    nc.tensor.transpose(
                        oT_ps[N : 2 * N, p, :],
                        osc[N : 2 * N, p, :],
                        ident2[N : 2 * N, :],
                    )
                if b == BPC - 1:
                    # fill the PE's wait on the Act-engine oT copy before the
                    # projection; osc is already written so these are ready
                    # immediately, and staying bf16 adds no dtype switch
                    for _ in range(2):
                        nc.tensor.matmul(
                            warm_ps[0:N, :], osc[0:N, 0, :],
                            warm_sb[0:N, 128:640], start=True, stop=True,
                        )
                oT_sb = epi.tile([128, CT, N], BF16)
                nc.scalar.copy(out=oT_sb[:], in_=oT_ps[:])
                out_ps = ps_epi.tile([N, C], F32, tag="epi_ps")
                for ct in range(CT):
                    nc.tensor.matmul(
                        out_ps[:],
                        oT_sb[:, ct, :],
                        wpt_sb[:, ct, :],
                        start=(ct == 0),
                        stop=(ct == CT - 1),
                    )
                out_sb = epi.tile([N, C], F32)
                nc.scalar.copy(out=out_sb[:], in_=out_ps[:])
                # DMA from the Activation queue: no cross-engine hop after
                # the copy that produced the data
                nc.scalar.dma_start(out=out_d[b], in_=out_sb[:])

    _split_multiwaits(nc)
    return nc


_NC_CACHE = None


def _get_nc():
    global _NC_CACHE
    if _NC_CACHE is None:
        _NC_CACHE = _build_kernel()
    return _NC_CACHE


def kernel(query, key, Wq, Wk, Wv, Wp, bp):
    global LAST_RESULT
    query = np.ascontiguousarray(query, dtype=np.float32)
    key = np.ascontiguousarray(key, dtype=np.float32)
    Wq = np.asarray(Wq, dtype=np.float32)
    Wk = np.asarray(Wk, dtype=np.float32)
    Wv = np.asarray(Wv, dtype=np.float32)
    Wp = np.asarray(Wp, dtype=np.float32)
    bp = np.asarray(bp, dtype=np.float32)

    # host prep: t[b,h,n,:] = Wk_h^T Wq_h query[b,n]  (tiny; never touches `key`)
    q = query @ Wq.T  # [B, N, C]
    qh = q.reshape(B, N, H, DH).transpose(0, 2, 1, 3)  # [B,H,N,DH]
    Wk_h = Wk.reshape(H, DH, C)
    t = np.einsum("bhnd,hdc->bhnc", qh, Wk_h)  # [B,H,N,C]
    # Tc[b] layout: [C, (h n)] with column h*N+n = t[b,h,n,:]
    Tc = np.ascontiguousarray(
        t.transpose(0, 3, 1, 2).reshape(B, C, H * N), dtype=np.float32
    )
    keyT = np.ascontiguousarray(key.transpose(0, 2, 1), dtype=np.float32)  # [B,C,S]
    WvT = np.ascontiguousarray(Wv.T, dtype=np.float32)
    WpT = np.ascontiguousarray(Wp.T).astype(ml_dtypes.bfloat16)
    # merged first transfer per core (batch 0 of that core): [kt chunk0 | tc]
    pre_all = [
        np.ascontiguousarray(
            np.concatenate([keyT[i * BPC][:, 0:128], Tc[i * BPC]], axis=1)
        )
        for i in range(NCORES)
    ]

    nc = _get_nc()
    in_maps = [
        {
            "pre": pre_all[i],
            "keyT": keyT[i * BPC : (i + 1) * BPC],
            "tc": Tc[i * BPC : (i + 1) * BPC],
            "wvt": WvT,
            "wpt": WpT,
        }
        for i in range(NCORES)
    ]
    try:
        res = run_bass_kernel_spmd(nc, in_maps, core_ids=list(range(NCORES)))
    except Exception:
        # transient NRT device errors have been observed; retry once
        res = run_bass_kernel_spmd(nc, in_maps, core_ids=list(range(NCORES)))
    LAST_RESULT = res
    out = np.concatenate([res.results[i]["out"] for i in range(NCORES)], axis=0)
    return (out + bp).astype(np.float32)



# revision 4
# speedup vs baseline: 1.0345x; 1.0345x over previous
"""Trainium2 Bass kernel for nn_AssignAttention (hard-assignment MoE-routing attention).

Math (forward): for each (b, h, key-token s), the key token is hard-assigned to
group n* = argmax_n (q_bhn . k_bhs); output per group = sum of assigned v vectors
scaled by 1/(count+1), then projected.  The straight-through softmax terms cancel
in forward up to ~1e-7, so only the argmax routing matters.

Strategy (v2, "P-scheme"):
 - Pure data-parallel over batch B=16 across 8 cores (2 batches/core), no collectives.
 - Host precomputes t[b,h,n,:] = Wk_h^T Wq_h query[b,n] so attention logits are
   attn[s, (h,n)] = key[b,s,:] . t[b,h,n,:]  -- one C-contraction against raw key.
 - v2 change: instead of computing v = key @ Wv^T per subtile and accumulating
   o += aT^T @ [v|1], accumulate the RAW-KEY group sums
       P[hn, c(+count)] += aT[s,hn]^T @ [key_bf16 | 1][s, c+1]
   (3 bf16 matmuls, 385-free, per 128-token subtile) and apply Wv ONCE per batch
   in the epilogue, folded into the existing transpose+Wp stage.  This removes
   per subtile: 3 f32r v-matmuls' weight loads, the 390-cycle o-matmul burst,
   and the 578ns Act-engine v65 copy.  Numerically validated on host:
   bf16 key + bf16 Wv with exact-f32 argmax gives rel err 0.0016 (flip-induced
   error from f32r logits dominates at ~0.015; gate is 2e-2).
 - Attention path unchanged: f32r keyT stationary / tc stream (fp16/bf16 logits
   measured 0.031/0.087 rel err -- dead), per-head argmax via DVE reduce_max +
   broadcast is_equal one-hot (both read PSUM, must stay on DVE).
 - key arrives twice: keyT f32r [c,s] for attn (12.6MB/core) and subtile-major
   bf16 [s,c+1] for P (6.3MB/core), the latter DMA'd from the otherwise-idle
   GpSimd sequencer so the latency-critical kt trigger path (Sync engine) is
   untouched.
 - P-flushes are batched one chunk behind (like v1's o-burst): 2 f32r<->bf16 PE
   reconfigurations per chunk instead of per subtile.  First matmul per P-bank
   per batch uses start=True so no memsets are needed.
 - Epilogue per batch: DVE scales P rows by 1/(count+1) straight out of PSUM
   (count rides in column 384 from the ones column of the key stream), 9 PE
   transposes [hn,c]->[c,hn], pair-packed Wv projection (9 matmuls, junk
   off-diagonal blocks never read), diagonal-block extraction on Act, then the
   v1 Wp projection + DMA out from the Act queue.
 - Startup ramp (merged first transfer, 128/128/256/512.. chunk schedule, PE
   warmup matmuls) kept from v1.
"""
import sys

sys.path.insert(0, "/opt/trn_rl_repo")

import numpy as np
import ml_dtypes

import concourse.bass as bass
import concourse.mybir as mybir
import concourse.tile as tile
from concourse.bass_utils import run_bass_kernel_spmd
from concourse.masks import make_identity

B, N, S, C, H = 16, 64, 4096, 384, 6
DH = C // H  # 64
NCORES = 8
BPC = B // NCORES  # batches per core = 2
CT = C // 128  # c-tiles = 3
NSUB = S // 128  # 32 subtiles per batch
# chunk boundaries: tiny leading chunks so the DMA pipeline can feed the PE
# as soon as the merged first transfer lands, then 512-token chunks
CHUNK_BOUNDS = [0, 128, 256, 512] + list(range(1024, S, 512)) + [S]
CHUNKS = list(zip(CHUNK_BOUNDS[:-1], CHUNK_BOUNDS[1:]))

F32 = mybir.dt.float32
F32R = mybir.dt.float32r
BF16 = mybir.dt.bfloat16

LAST_RESULT = None  # stash of BassKernelResults for profiling in test.py


def _split_multiwaits(nc):
    """walrus codegen in this toolchain accepts at most one sync-wait per
    instruction; hoist extras onto standalone wait-only EventSemaphore
    instructions placed immediately before (same engine, so ordering holds)."""
    for fn in nc.m.functions:
        for blk in fn.blocks:
            new = []
            for inst in blk.instructions:
                si = inst.sync_info
                if si is not None and si.on_wait and len(si.on_wait) > 1:
                    for w in si.on_wait[:-1]:
                        ev = mybir.InstEventSemaphore(
                            name=nc.get_next_instruction_name(), ins=[], outs=[]
                        )
                        ev.engine = inst.engine
                        ev.sync_info = mybir.SyncInfo(on_wait=[w], on_update=[])
                        new.append(ev)
                    inst.sync_info = mybir.SyncInfo(
                        on_wait=[si.on_wait[-1]], on_update=si.on_update
                    )
                new.append(inst)
            blk.instructions = new


def _build_kernel():
    nc = bass.Bass()
    # pre: merged [kt chunk0 | tc] for batch 0; row (ct*128+p) = [key tokens
    # 0:128 | tc columns] of c-row ct*128+p, so each (p, ct) descriptor is 2KB
    pre_d = nc.declare_dram_parameter("pre", [C, 128 + C], F32R, isOutput=False)
    keyT_d = nc.declare_dram_parameter("keyT", [BPC, C, S], F32R, isOutput=False)
    tc_d = nc.declare_dram_parameter("tc", [BPC, C, C], F32R, isOutput=False)
    # key65: subtile-major bf16 raw key with a ones column for the counts;
    # [b, p, sub, x] = key[b, sub*128+p, x] (x==384 -> 1.0)
    key65_d = nc.declare_dram_parameter(
        "key65", [BPC, 128, NSUB, C + 1], BF16, isOutput=False
    )
    wvt_d = nc.declare_dram_parameter("wvt", [C, C], BF16, isOutput=False)
    wpt_d = nc.declare_dram_parameter("wpt", [C, C], BF16, isOutput=False)
    out_d = nc.declare_dram_parameter("out", [BPC, N, C], F32, isOutput=True)

    with tile.TileContext(nc) as tc:
        with (
            tc.tile_pool(name="consts", bufs=1) as consts,
            tc.tile_pool(name="perb", bufs=2) as perb,
            tc.tile_pool(name="keyp", bufs=6) as keyp,
            tc.tile_pool(name="k65p", bufs=4) as k65p,
            tc.tile_pool(name="work", bufs=1) as work,
            tc.tile_pool(name="epi", bufs=2) as epi,
            tc.tile_pool(name="ps_attn", bufs=4, space="PSUM") as ps_attn,
            tc.tile_pool(name="ps_P", bufs=3, space="PSUM") as ps_P,
            tc.tile_pool(name="ps_epi", bufs=1, space="PSUM") as ps_epi,
        ):
            # one merged transfer delivers everything subtile 0 needs
            pre_sb = consts.tile([128, CT, 128 + C], F32R)
            nc.sync.dma_start(
                out=pre_sb[:],
                in_=pre_d.rearrange("(ct p) x -> p ct x", p=128),
            )
            kt_c0 = pre_sb[:, :, 0:128]
            tc_b0 = pre_sb[:, :, 128 : 128 + C]
            keyT_b0 = keyT_d[0].rearrange("(ct p) s -> p ct s", p=128)
            s0, s1 = CHUNKS[1]
            kt_c1 = keyp.tile([128, CT, s1 - s0], F32R, tag="kt")
            nc.sync.dma_start(out=kt_c1[:], in_=keyT_b0[:, :, s0:s1])
            s0, s1 = CHUNKS[2]
            kt_c2 = keyp.tile([128, CT, s1 - s0], F32R, tag="kt")
            nc.sync.dma_start(out=kt_c2[:], in_=keyT_b0[:, :, s0:s1])
            # wvt/wpt are needed only at the first epilogue (~40us in), so
            # their transfers are deferred out of the latency-critical early
            # DMA FIFO (triggers emitted after chunk 5's, inside the loop)
            wvt_sb = consts.tile([128, CT, C], BF16)  # [c_in_p, ct, (h d)]
            wpt_sb = consts.tile([128, CT, C], BF16)  # [hd_p, ct, c_out]
            # identities: 128x128 for the epilogue [hn,c]->[c,hn] transposes
            ident = consts.tile([128, 128], BF16)
            make_identity(nc, ident[:])

            # PE warmup: back-to-back matmuls on scratch while the first
            # transfer lands, so the pstate ramp completes before real work.
            # The psum bank is never read; its reuse starts with start=True.
            warm_sb = consts.tile([128, 640], BF16)
            nc.gpsimd.memset(warm_sb[:], 0.0)
            warm_ps = ps_attn.tile([128, 512], F32, tag="attn_ps")
            for _ in range(10):
                nc.tensor.matmul(
                    warm_ps[:], warm_sb[:, 0:128], warm_sb[:, 128:640],
                    start=True, stop=True,
                )

            for b in range(BPC):
                if b == 0:
                    tc_sb = tc_b0
                else:
                    tc_t = perb.tile([128, CT, C], F32R, tag="tc_sb")
                    nc.sync.dma_start(
                        out=tc_t[:],
                        in_=tc_d[b].rearrange("(ct p) hn -> p ct hn", p=128),
                    )
                    tc_sb = tc_t[:, :, :]
                # raw-key group-sum accumulators: P[p] rows = hn-slice p
                # (heads 2p, 2p+1), cols 0:384 = summed bf16 key, col 384 =
                # count.  No memset: the first flush per bank uses start=True.
                P_ps = [
                    ps_P.tile([128, C + 1], F32, tag="P", name=f"P_{b}_{p}")
                    for p in range(CT)
                ]
                p_started = [False] * CT

                keyT_b = keyT_d[b].rearrange("(ct p) s -> p ct s", p=128)
                # P-matmuls are flushed one chunk at a time, after the NEXT
                # chunk's first subtile's attn (see module docstring)
                pending = []  # [(aT, key65_ap), ...] of the previous chunk

                def flush_P(stop):
                    for i, (aT_p, k65_ap) in enumerate(pending):
                        last_sub = i == len(pending) - 1
                        for p in range(CT):
                            nc.tensor.matmul(
                                P_ps[p][:],
                                aT_p[:].rearrange("q h n -> q (h n)")[
                                    :, p * 128 : (p + 1) * 128
                                ],
                                k65_ap,
                                start=not p_started[p],
                                stop=stop and last_sub,
                                skip_group_check=True,
                            )
                            p_started[p] = True
                    pending.clear()

                for ci, (s0, s1) in enumerate(CHUNKS):
                    n0, n1 = s0 // 128, s1 // 128
                    if b == 0 and ci == 0:
                        kt_sb = kt_c0
                    elif b == 0 and ci == 1:
                        kt_sb = kt_c1[:, :, :]
                    elif b == 0 and ci == 2:
                        kt_sb = kt_c2[:, :, :]
                    else:
                        kt_t = keyp.tile([128, CT, s1 - s0], F32R, tag="kt")
                        nc.sync.dma_start(
                            out=kt_t[:], in_=keyT_b[:, :, s0:s1]
                        )
                        kt_sb = kt_t[:, :, :]
                        if b == 0 and ci == 5:
                            nc.sync.dma_start(
                                out=wvt_sb[:],
                                in_=wvt_d.rearrange("(ct p) co -> p ct co", p=128),
                            )
                            nc.sync.dma_start(
                                out=wpt_sb[:],
                                in_=wpt_d.rearrange("(ct p) co -> p ct co", p=128),
                            )
                    # bf16 [s, c|1] stream for this chunk's P-flush (needed
                    # only one chunk later; issued from the idle GpSimd
                    # sequencer to keep Sync's kt trigger path clean)
                    k65_t = k65p.tile([128, n1 - n0, C + 1], BF16, tag="k65")
                    nc.gpsimd.dma_start(
                        out=k65_t[:], in_=key65_d[b, :, n0:n1, :]
                    )
                    carry = []
                    for sub in range(n1 - n0):
                        sl = slice(sub * 128, (sub + 1) * 128)
                        attn_ps = ps_attn.tile([128, C], F32)
                        for ct in range(CT):
                            nc.tensor.matmul(
                                attn_ps[:],
                                kt_sb[:, ct, sl],
                                tc_sb[:, ct, :],
                                start=(ct == 0),
                                stop=(ct == CT - 1),
                            )
                        if sub == min(1, n1 - n0 - 1) and pending:
                            # flush the previous chunk's P-burst one subtile
                            # later than strictly needed: the extra subtile of
                            # DVE slack hides the last one-hot's latency so
                            # the burst never stalls on entry
                            flush_P(stop=False)
                        # per-head argmax -> one-hot (bf16); both ops read
                        # PSUM so they must stay on DVE (GpSimd cannot
                        # access PSUM)
                        gmax = work.tile([128, H], F32, tag="gmax", bufs=4)
                        nc.vector.reduce_max(
                            out=gmax[:],
                            in_=attn_ps[:].rearrange("p (h n) -> p h n", h=H),
                            axis=mybir.AxisListType.X,
                        )
                        aT = work.tile([128, H, N], BF16, tag="aT", bufs=12)
                        g = gmax[:]
                        g_bcast = bass.AP(
                            tensor=g.tensor, offset=g.offset,
                            ap=[g.ap[0], g.ap[1], [0, N]],
                        )
                        nc.vector.tensor_tensor(
                            out=aT[:],
                            in0=attn_ps[:].rearrange("p (h n) -> p h n", h=H),
                            in1=g_bcast,
                            op=mybir.AluOpType.is_equal,
                        )
                        carry.append((aT, k65_t[:, sub, :]))
                    pending.extend(carry)
                last_aT = pending[-1][0]
                flush_P(stop=True)
                if b == BPC - 1:
                    # fill the PE's wait on DVE's scaling chain with scratch
                    # matmuls so its clock stays in the top p-state for the
                    # tail transposes/projection.  Reading the final one-hot
                    # keeps the scheduler from hoisting them earlier, and
                    # bf16 operands right after the bf16 P-burst add no
                    # dtype-switch.
                    warm_w = last_aT[:].rearrange("q h n -> q (h n)")[:, 0:128]
                    for _ in range(3):
                        nc.tensor.matmul(
                            warm_ps[:], warm_w, warm_sb[:, 128:640],
                            start=True, stop=True,
                        )
                # ---- epilogue for this b ----
                # 1/(cnt+1) from P col 384, then scale rows 0:384 out of PSUM
                scl = epi.tile([128, CT], F32)
                for p in range(CT):
                    nc.vector.tensor_scalar(
                        out=scl[:, p : p + 1],
                        in0=P_ps[p][:, C : C + 1],
                        scalar1=1.0,
                        scalar2=None,
                        op0=mybir.AluOpType.add,
                    )
                nc.vector.reciprocal(out=scl[:], in_=scl[:])
                osc = epi.tile([128, CT, C], BF16)  # [hn-slice rows, p, c]
                for p in range(CT):
                    sp = scl[:, p : p + 1]
                    spb = bass.AP(
                        tensor=sp.tensor, offset=sp.offset,
                        ap=[sp.ap[0], [0, C]],
                    )
                    nc.vector.tensor_tensor(
                        out=osc[:, p, :],
                        in0=P_ps[p][:, 0:C],
                        in1=spb,
                        op=mybir.AluOpType.mult,
                    )
                # transpose [hn, c] -> [c, hn]: 9 PE transposes of 128x128
                # bf16 blocks; outputs per-ct into recycled P banks
                oscT_sb = epi.tile([128, CT, C], BF16)  # [c_in_p, ct, hn]
                for ct in range(CT):
                    oscT_ps = ps_P.tile([128, C], BF16, tag="P")
                    for p in range(CT):
                        nc.tensor.transpose(
                            oscT_ps[:, p * 128 : (p + 1) * 128],
                            osc[:, p, ct * 128 : (ct + 1) * 128],
                            ident[:],
                        )
                    nc.scalar.copy(out=oscT_sb[:, ct, :], in_=oscT_ps[:])
                # pair-packed Wv projection: o2[p] = [hd-slice p, n-pack]
                # (off-diagonal head-cross blocks are junk, never read)
                o2_ps = ps_epi.tile([128, CT, 128], F32, tag="epi_ps")
                for p in range(CT):
                    for ct in range(CT):
                        nc.tensor.matmul(
                            o2_ps[:, p, :],
                            wvt_sb[:, ct, p * 128 : (p + 1) * 128],
                            oscT_sb[:, ct, p * 128 : (p + 1) * 128],
                            start=(ct == 0),
                            stop=(ct == CT - 1),
                            skip_group_check=True,
                        )
                # extract the two diagonal 64x64 blocks per pair -> oT [hd, n]
                oT_sb = epi.tile([128, CT, N], BF16)
                for p in range(CT):
                    nc.scalar.copy(
                        out=oT_sb[0:N, p, :], in_=o2_ps[0:N, p, 0:N]
                    )
                    nc.scalar.copy(
                        out=oT_sb[N : 2 * N, p, :],
                        in_=o2_ps[N : 2 * N, p, N : 2 * N],
                    )
                if b == BPC - 1:
                    # fill the PE's wait on the Act-engine diag copies before
                    # the projection; osc is already written so these are
                    # ready immediately, and staying bf16 adds no dtype switch
                    for _ in range(2):
                        nc.tensor.matmul(
                            warm_ps[0:N, :], osc[0:N, 0, 0:N],
                            warm_sb[0:N, 128:640], start=True, stop=True,
                        )
                out_ps = ps_epi.tile([N, C], F32, tag="epi_ps")
                for ct in range(CT):
                    nc.tensor.matmul(
                        out_ps[:],
                        oT_sb[:, ct, :],
                        wpt_sb[:, ct, :],
                        start=(ct == 0),
                        stop=(ct == CT - 1),
                    )
                out_sb = epi.tile([N, C], F32)
                nc.scalar.copy(out=out_sb[:], in_=out_ps[:])
                # DMA from the Activation queue: no cross-engine hop after
                # the copy that produced it
                nc.scalar.dma_start(out=out_d[b], in_=out_sb[:])

    _split_multiwaits(nc)
    return nc


_NC_CACHE = None


def _get_nc():
    global _NC_CACHE
    if _NC_CACHE is None:
        _NC_CACHE = _build_kernel()
    return _NC_CACHE


def kernel(query, key, Wq, Wk, Wv, Wp, bp):
    global LAST_RESULT
    query = np.ascontiguousarray(query, dtype=np.float32)
    key = np.ascontiguousarray(key, dtype=np.float32)
    Wq = np.asarray(Wq, dtype=np.float32)
    Wk = np.asarray(Wk, dtype=np.float32)
    Wv = np.asarray(Wv, dtype=np.float32)
    Wp = np.asarray(Wp, dtype=np.float32)
    bp = np.asarray(bp, dtype=np.float32)

    # host prep: t[b,h,n,:] = Wk_h^T Wq_h query[b,n]  (tiny; never touches `key`)
    q = query @ Wq.T  # [B, N, C]
    qh = q.reshape(B, N, H, DH).transpose(0, 2, 1, 3)  # [B,H,N,DH]
    Wk_h = Wk.reshape(H, DH, C)
    t = np.einsum("bhnd,hdc->bhnc", qh, Wk_h)  # [B,H,N,C]
    # Tc[b] layout: [C, (h n)] with column h*N+n = t[b,h,n,:]
    Tc = np.ascontiguousarray(
        t.transpose(0, 3, 1, 2).reshape(B, C, H * N), dtype=np.float32
    )
    keyT = np.ascontiguousarray(key.transpose(0, 2, 1), dtype=np.float32)  # [B,C,S]
    # subtile-major bf16 key with ones column: [B, 128, S/128, C+1]
    key65 = np.empty((B, S, C + 1), dtype=ml_dtypes.bfloat16)
    key65[:, :, 0:C] = key.astype(ml_dtypes.bfloat16)
    key65[:, :, C] = 1.0
    key65 = np.ascontiguousarray(
        key65.reshape(B, NSUB, 128, C + 1).transpose(0, 2, 1, 3)
    )
    WvT = np.ascontiguousarray(Wv.T).astype(ml_dtypes.bfloat16)
    WpT = np.ascontiguousarray(Wp.T).astype(ml_dtypes.bfloat16)
    # merged first transfer per core (batch 0 of that core): [kt chunk0 | tc]
    pre_all = [
        np.ascontiguousarray(
            np.concatenate([keyT[i * BPC][:, 0:128], Tc[i * BPC]], axis=1)
        )
        for i in range(NCORES)
    ]

    nc = _get_nc()
    in_maps = [
        {
            "pre": pre_all[i],
            "keyT": keyT[i * BPC : (i + 1) * BPC],
            "tc": Tc[i * BPC : (i + 1) * BPC],
            "key65": key65[i * BPC : (i + 1) * BPC],
            "wvt": WvT,
            "wpt": WpT,
        }
        for i in range(NCORES)
    ]
    try:
        res = run_bass_kernel_spmd(nc, in_maps, core_ids=list(range(NCORES)))
    except Exception:
        # transient NRT device errors have been observed; retry once
        res = run_bass_kernel_spmd(nc, in_maps, core_ids=list(range(NCORES)))
    LAST_RESULT = res
    out = np.concatenate([res.results[i]["out"] for i in range(NCORES)], axis=0)
    return (out + bp).astype(np.float32)
